# revision 20
# baseline (speedup 1.0000x reference)
"""Llama-style transformer block on 8 TRN2 NeuronCores.

v5: skew-tolerant scheduling.  The machine's FIFO queues (scalar/sync/
gpsimd engine queues, cc queue) must never sit waiting on a collective
whose peers are still computing -- everything that CONSUMES a collective
result is emitted at a program point whose wall-clock is safely after the
collective completes, even with ~30us cross-core skew.
  - Stage A: per-chunk x-stats partials (own 512 dims, 4 ones-matmuls)
    + tiny AllReduce; x quarters on gpsimd queue, wqk alternated over
    scalar/sync queues (per-queue DMA bandwidth is the stage A pacer).
  - Attention: scores/rowsum staggered one ktile apart; AV chain after the
    rowsum chain so recip/broadcast hide under it; og casts on DVE only.
  - wo ROW-sharded (no attnT AllGather); wo_mm(c) -> RS(o_c).
  - h_block(c) (RS readback + h + h^2 stats matmuls + AG(h) + AR(ssh))
    and s2chain(c) (AR readback -> sqrt -> recip -> broadcast) are
    scheduled 1-2 chunks behind the RS/AR they consume.
  - FFN: hn pre-scaled once (bf16), silu straight from PSUM,
    g = silu(p1)*p3; weight loads split across scalar+sync queues.
Program order: A0..A3 | B0 B1 wo0 B2 wo1 h0 B3 wo2 h1 wo3 h2 s2c0 |
ffn0[hn,scale,s2c1,ft-loop,h3,s2c2,w2,RS,res] ffn1[...,s2c3,...] ffn2 ffn3
"""

import math

import ml_dtypes
import numpy as np

import concourse.bass as bass
import concourse.mybir as mybir
import concourse.tile as tile
from concourse import bacc
from concourse.bass_utils import run_bass_kernel_spmd

S = 2048
D = 4096
HD = 128
NH = 32
F = 11008
CORES = 8
NHC = NH // CORES          # heads per core = 4
DQ = NHC * HD              # q/k/v dims per core = 512
FC = F // CORES            # ffn dims per core = 1376
FT = 11                    # padded f-tiles per core
FP = FT * 128
EPS = 1e-5
P = 128
NCH = 4                    # 512-token chunks
CW = S // NCH              # chunk width = 512
DT = D // P                # d tiles = 32
ST = S // P                # s tiles = 16

CDT = mybir.dt.bfloat16
NP_CDT = ml_dtypes.bfloat16

_COMPILED = None


def _build():
    nc = bacc.Bacc("TRN2", target_bir_lowering=False, debug=False,
                   num_devices=CORES)
    f32 = mybir.dt.float32

    # ---- kernel I/O ----
    xT_s = nc.declare_dram_parameter("xT_s", [DQ, S], f32, isOutput=False)
    x_ch = nc.declare_dram_parameter("x_ch", [NCH, P, DT, CW], CDT,
                                     isOutput=False)
    w_qk = nc.declare_dram_parameter("w_qk", [8, P, DT, P], CDT, isOutput=False)
    w_v = nc.declare_dram_parameter("w_v", [DT, P, DQ], CDT, isOutput=False)
    w_o = nc.declare_dram_parameter("w_o", [DT, P, NHC, P], CDT, isOutput=False)
    w_1 = nc.declare_dram_parameter("w_1", [FT, P, DT, P], CDT, isOutput=False)
    w_3 = nc.declare_dram_parameter("w_3", [FT, P, DT, P], CDT, isOutput=False)
    w_2 = nc.declare_dram_parameter("w_2", [32, P, FT, P], CDT, isOutput=False)
    cos2 = nc.declare_dram_parameter("cos2", [P, S], CDT, isOutput=False)
    sinsg2 = nc.declare_dram_parameter("sinsg2", [P, S], CDT, isOutput=False)
    dmask = nc.declare_dram_parameter("dmask", [P, P], f32, isOutput=False)
    outT_s = nc.declare_dram_parameter("outT_s", [DQ, S], f32, isOutput=True)

    # ---- internal DRAM ----
    s1row = nc.dram_tensor("s1row", [1, S], f32)
    ssx_p = [nc.dram_tensor(f"ssx_p{c}", [1, CW], f32) for c in range(NCH)]
    ssx_a = [nc.dram_tensor(f"ssx_a{c}", [1, CW], f32) for c in range(NCH)]
    ssh_p = [nc.dram_tensor(f"ssh_p{c}", [1, CW], f32) for c in range(NCH)]
    ssh_a = [nc.dram_tensor(f"ssh_a{c}", [1, CW], f32) for c in range(NCH)]
    oT_cc = [nc.dram_tensor(f"oT_cc{c}", [D, CW], CDT) for c in range(NCH)]
    o_rs = [nc.dram_tensor(f"o_rs{c}", [DQ, CW], CDT) for c in range(NCH)]
    h_cc = [nc.dram_tensor(f"h_cc{c}", [DQ, CW], CDT) for c in range(NCH)]
    hT_ag = [nc.dram_tensor(f"hT_ag{c}", [D, CW], CDT, addr_space="Shared")
             for c in range(NCH)]
    foT_cc = [nc.dram_tensor(f"foT_cc{c}", [D, CW], CDT) for c in range(NCH)]
    fo_rs = [nc.dram_tensor(f"fo_rs{c}", [DQ, CW], CDT) for c in range(NCH)]

    RG = [list(range(CORES))]
    ADD = mybir.AluOpType.add
    BYP = mybir.AluOpType.bypass
    EXP = mybir.ActivationFunctionType.Exp
    SQRT = mybir.ActivationFunctionType.Sqrt
    SILU = mybir.ActivationFunctionType.Silu
    ISQ = 1.0 / math.sqrt(HD)

    def ch(c):
        return slice(CW * c, CW * (c + 1))

    with tile.TileContext(nc) as tc:
        with (
            tc.tile_pool(name="persist", bufs=1) as persist,
        ):
            ones = persist.tile([P, 1], CDT)
            nc.vector.memset(ones[:], 1.0)
            eps_sb = persist.tile([P, 1], f32)
            nc.vector.memset(eps_sb[:], EPS)
            dmask_sb = persist.tile([P, P], f32)
            nc.gpsimd.dma_start(out=dmask_sb[:], in_=dmask[:])
            hT = [persist.tile([P, S], CDT, tag=f"hT{i}", name=f"hT{i}")
                  for i in range(4)]
            s2rep = persist.tile([P, S], CDT)
            s1tok = persist.tile([P, ST], f32)

            with tc.tile_pool(name="qkvsb", bufs=1) as qkvsb:
                qts = [qkvsb.tile([P, S], CDT, tag=f"qt{h}", name=f"qt{h}")
                       for h in range(NHC)]
                kts = [qkvsb.tile([P, S], CDT, tag=f"kt{h}", name=f"kt{h}")
                       for h in range(NHC)]
                v_sb = qkvsb.tile([P, ST, DQ], CDT)

                # ======== stage A: local stats + Q/K/V (+RoPE) ========
                with (
                    tc.tile_pool(name="tbl", bufs=1) as tbl,
                    tc.tile_pool(name="xst1", bufs=1) as xst1,
                    tc.tile_pool(name="xst", bufs=2) as xst,
                    tc.tile_pool(name="stAx", bufs=5) as stAx,
                    tc.tile_pool(name="stAw", bufs=2) as stAw,
                    tc.tile_pool(name="wvp", bufs=1) as wvp,
                    tc.tile_pool(name="rope", bufs=2) as rope,
                    tc.tile_pool(name="ps_qkv", bufs=3, space="PSUM") as ps_qkv,
                    tc.tile_pool(name="ps_v", bufs=1, space="PSUM") as ps_v,
                    tc.tile_pool(name="ps_xst", bufs=1, space="PSUM") as ps_xst,
                ):
                    # chunk-0 x quarters first so the first matmul is early
                    xq_c0 = [stAx.tile([P, 8, CW], CDT, tag="xq",
                                       name=f"xq{j}_0") for j in range(4)]
                    for j in range(4):
                        nc.gpsimd.dma_start(
                            out=xq_c0[j][:], in_=x_ch[0][:, 8 * j:8 * (j + 1), :])
                    cos_raw = tbl.tile([P, S], CDT, tag="cosr")
                    sin_raw = tbl.tile([P, S], CDT, tag="sinr")
                    nc.gpsimd.dma_start(out=cos_raw[:], in_=cos2[:])
                    nc.gpsimd.dma_start(out=sin_raw[:], in_=sinsg2[:])
                    wv_sb = wvp.tile([P, DT, DQ], CDT)

                    tables = {}

                    def stats_block(c):
                        # partial ssq over own 512 dims -> tiny AllReduce
                        xsl = xst1.tile([P, 4, CW], f32, tag="xsl",
                                        name=f"xsl{c}")
                        for i in range(4):
                            nc.gpsimd.dma_start(
                                out=xsl[:, i, :],
                                in_=xT_s[P * i:P * (i + 1), ch(c)])
                        sqx = xst1.tile([P, 4, CW], CDT, tag="sqx",
                                        name=f"sqx{c}")
                        nc.vector.tensor_mul(sqx[:], xsl[:], xsl[:])
                        pst = ps_xst.tile([1, CW], f32, tag="pst")
                        for i in range(4):
                            nc.tensor.matmul(pst[:], ones[:], sqx[:, i, :],
                                             start=(i == 0), stop=(i == 3))
                        row = xst.tile([1, CW], f32, tag="xrow",
                                       name=f"xrow{c}")
                        nc.scalar.copy(out=row[:], in_=pst[:])
                        nc.gpsimd.dma_start(out=ssx_p[c][:], in_=row[:])
                        nc.gpsimd.collective_compute(
                            "AllReduce", ADD, ins=[ssx_p[c][:]],
                            outs=[ssx_a[c][:]], replica_groups=RG)
                        rowa = xst.tile([1, CW], f32, tag="xrowa",
                                        name=f"xrowa{c}")
                        nc.gpsimd.dma_start(out=rowa[:], in_=ssx_a[c][:])
                        nc.scalar.activation(out=rowa[:], in_=rowa[:],
                                             func=SQRT, bias=eps_sb[0:1],
                                             scale=1.0 / D)
                        nc.vector.reciprocal(out=rowa[:], in_=rowa[:])
                        # token-major copy for the V scale
                        nc.gpsimd.dma_start(out=s1row[0:1, ch(c)], in_=rowa[:])
                        nc.gpsimd.dma_start(
                            out=s1tok[:, 4 * c:4 * c + 4],
                            in_=s1row[0:1, ch(c)].rearrange(
                                "o (j p) -> p (o j)", p=P))
                        s1b = xst.tile([1, CW], CDT, tag="s1b", name=f"s1b{c}")
                        nc.vector.tensor_copy(out=s1b[:], in_=rowa[:])
                        s1rep = xst.tile([P, CW], CDT, tag="s1rep",
                                         name=f"s1rep{c}")
                        nc.gpsimd.partition_broadcast(s1rep[:], s1b[:])
                        cs_t = rope.tile([P, CW], CDT, tag="cs", name=f"cs{c}")
                        sn_t = rope.tile([P, CW], CDT, tag="sn", name=f"sn{c}")
                        nc.vector.tensor_mul(cs_t[:], cos_raw[:, ch(c)],
                                             s1rep[:])
                        nc.vector.tensor_mul(sn_t[:], sin_raw[:, ch(c)],
                                             s1rep[:])
                        tables[c] = (cs_t, sn_t)

                    stats_block(0)

                    for c in range(NCH):
                      with nc.named_scope(f"qkv_c{c}"):
                        if c == 0:
                            xq = xq_c0
                        else:
                            xq = [stAx.tile([P, 8, CW], CDT, tag="xq",
                                            name=f"xq{j}_{c}")
                                  for j in range(4)]
                            for j in range(4):
                                nc.gpsimd.dma_start(
                                    out=xq[j][:],
                                    in_=x_ch[c][:, 8 * j:8 * (j + 1), :])

                        def xkt(kt):
                            return xq[kt // 8][:, kt % 8, :]

                        cs_t, sn_t = tables.pop(c)
                        if c + 1 < NCH:
                            stats_block(c + 1)  # pipelined under this chunk

                        # --- Q and K projections + RoPE ---
                        for ot in range(8):
                            if c == 0 and ot == 5:
                                # first wv half slots into the sync queue
                                # between the odd wqk loads of chunk 0
                                nc.sync.dma_start(
                                    out=wv_sb[:, 0:16, :],
                                    in_=w_v[0:16].rearrange("k p q -> p k q"))
                            wt = stAw.tile([P, DT, P], CDT, tag="wqk")
                            if ot % 2 == 0:
                                nc.scalar.dma_start(out=wt[:], in_=w_qk[ot])
                            else:
                                nc.sync.dma_start(out=wt[:], in_=w_qk[ot])
                            pt = ps_qkv.tile([P, CW], f32, tag="pqk")
                            for kt in range(DT):
                                nc.tensor.matmul(pt[:], wt[:, kt], xkt(kt),
                                                 start=(kt == 0),
                                                 stop=(kt == DT - 1))
                            swp = rope.tile([P, CW], f32, tag="swp")
                            nc.vector.tensor_copy(swp[0:64, :], pt[64:128, :])
                            nc.vector.tensor_copy(swp[64:128, :], pt[0:64, :])
                            t1 = rope.tile([P, CW], f32, tag="t1")
                            nc.vector.tensor_mul(t1[:], pt[:], cs_t[:])
                            nc.vector.tensor_mul(swp[:], swp[:], sn_t[:])
                            dst = qts[ot % 4] if ot < 4 else kts[ot % 4]
                            nc.vector.tensor_add(dst[:, ch(c)], t1[:], swp[:])

                        # --- V: 4 token-tiles of this chunk ---
                        if c == 0:
                            nc.sync.dma_start(
                                out=wv_sb[:, 16:32, :],
                                in_=w_v[16:32].rearrange("k p q -> p k q"))
                        pts = [ps_v.tile([P, DQ], f32, tag=f"pv{i}",
                                         name=f"pv{i}") for i in range(4)]
                        for kt in range(DT):
                            for i in range(4):
                                tok = slice(P * i, P * (i + 1))
                                nc.tensor.matmul(
                                    pts[i][:], xkt(kt)[:, tok],
                                    wv_sb[:, kt, :],
                                    start=(kt == 0), stop=(kt == DT - 1))
                        for i in range(4):
                            st = 4 * c + i
                            nc.vector.tensor_scalar_mul(
                                out=v_sb[:, st, :], in0=pts[i][:],
                                scalar1=s1tok[:, st:st + 1])

                def h_block(c, hst, xtp, ps_hst):
                    # RS(o_c) must be complete (with skew margin) at the
                    # wall-clock of this block's program position.
                    with nc.named_scope(f"h_c{c}"):
                        psh = ps_hst.tile([1, CW], f32, tag="psh")
                        for i in range(4):
                            osb = hst.tile([P, CW], CDT, tag="osb")
                            nc.gpsimd.dma_start(
                                out=osb[:], in_=o_rs[c][P * i:P * (i + 1), :])
                            xt = xtp.tile([P, CW], f32, tag="xt")
                            nc.sync.dma_start(
                                out=xt[:], in_=xT_s[P * i:P * (i + 1), ch(c)])
                            nc.vector.tensor_add(hT[i][:, ch(c)], xt[:],
                                                 osb[:])
                            nc.gpsimd.dma_start(
                                out=h_cc[c][:].rearrange(
                                    "(p k) s -> p k s", p=P)[:, i, :],
                                in_=hT[i][:, ch(c)])
                            sq = hst.tile([P, CW], CDT, tag="sq")
                            nc.vector.tensor_mul(sq[:], hT[i][:, ch(c)],
                                                 hT[i][:, ch(c)])
                            nc.tensor.matmul(psh[:], ones[:], sq[:],
                                             start=(i == 0), stop=(i == 3))
                        nc.gpsimd.collective_compute(
                            "AllGather", BYP, ins=[h_cc[c][:]],
                            outs=[hT_ag[c][:]], replica_groups=RG)
                        hrow = hst.tile([1, CW], f32, tag="hrow")
                        nc.scalar.copy(out=hrow[:], in_=psh[:])
                        nc.gpsimd.dma_start(out=ssh_p[c][:], in_=hrow[:])
                        nc.gpsimd.collective_compute(
                            "AllReduce", ADD, ins=[ssh_p[c][:]],
                            outs=[ssh_a[c][:]], replica_groups=RG)

                def s2chain(c, hst):
                    # AR(ssh_c) must be complete at this program position.
                    with nc.named_scope(f"s2_c{c}"):
                        rh = hst.tile([1, CW], f32, tag="s2r")
                        nc.gpsimd.dma_start(out=rh[:], in_=ssh_a[c][:])
                        nc.scalar.activation(out=rh[:], in_=rh[:], func=SQRT,
                                             bias=eps_sb[0:1], scale=1.0 / D)
                        nc.vector.reciprocal(out=rh[:], in_=rh[:])
                        s2b = hst.tile([1, CW], CDT, tag="s2b")
                        nc.vector.tensor_copy(out=s2b[:], in_=rh[:])
                        nc.gpsimd.partition_broadcast(s2rep[:, ch(c)], s2b[:])

                # ======== stage B: attention + row-sharded wo ========
                if True:
                    with (
                        tc.tile_pool(name="stB", bufs=4) as stB,
                        tc.tile_pool(name="exps", bufs=18) as exps,
                        tc.tile_pool(name="attp", bufs=8) as attp,
                        tc.tile_pool(name="woW", bufs=1) as woW,
                        tc.tile_pool(name="hstB", bufs=2) as hstB,
                        tc.tile_pool(name="xtpB", bufs=6) as xtpB,
                        tc.tile_pool(name="ps_sc", bufs=2,
                                     space="PSUM") as ps_sc,
                        tc.tile_pool(name="ps_av", bufs=2,
                                     space="PSUM") as ps_av,
                        tc.tile_pool(name="ps_sm", bufs=1,
                                     space="PSUM") as ps_sm,
                        tc.tile_pool(name="ps_wo", bufs=2,
                                     space="PSUM") as ps_wo,
                        tc.tile_pool(name="ps_hstB", bufs=1,
                                     space="PSUM") as ps_hstB,
                    ):
                        wo_sb = [woW.tile([P, NHC, P], CDT, tag=f"wo{ot}",
                                          name=f"wo{ot}") for ot in range(DT)]
                        for ot in range(DT):
                            nc.sync.dma_start(out=wo_sb[ot][:], in_=w_o[ot])

                        at_ts = {}

                        def attn_chunk(qc):
                          with nc.named_scope(f"attn_c{qc}"):
                            nkt = 4 * qc + 4
                            for hh in range(NHC):
                                qt, kt_t = qts[hh], kts[hh]
                                smp = ps_sm.tile([1, CW], f32, tag="smp")
                                ets = []
                                for ktile in range(nkt):
                                    diag = ktile >= 4 * qc
                                    col0 = P * (ktile - 4 * qc) if diag else 0
                                    scp = ps_sc.tile([P, CW], f32, tag="scp")
                                    nc.tensor.matmul(
                                        scp[:, col0:],
                                        kt_t[:, P * ktile:P * (ktile + 1)],
                                        qt[:, CW * qc + col0:CW * (qc + 1)],
                                        start=True, stop=True)
                                    if diag:
                                        nc.vector.tensor_add(
                                            scp[:, col0:col0 + P],
                                            scp[:, col0:col0 + P],
                                            dmask_sb[:])
                                    et = exps.tile([P, CW], CDT, tag="et")
                                    nc.scalar.activation(out=et[:, col0:],
                                                         in_=scp[:, col0:],
                                                         func=EXP, scale=ISQ)
                                    ets.append((et, col0))
                                    if ktile > 0:
                                        pe, pc0 = ets[ktile - 1]
                                        nc.tensor.matmul(
                                            smp[:, pc0:], ones[:],
                                            pe[:, pc0:],
                                            start=(ktile == 1), stop=False)
                                pe, pc0 = ets[nkt - 1]
                                nc.tensor.matmul(smp[:, pc0:], ones[:],
                                                 pe[:, pc0:],
                                                 start=(nkt == 1), stop=True)
                                rec = stB.tile([1, CW], f32, tag="rec")
                                nc.vector.reciprocal(out=rec[:], in_=smp[:])
                                avp = ps_av.tile([P, CW], f32, tag="avp")
                                for ktile in range(nkt):
                                    et, col0 = ets[ktile]
                                    nc.tensor.matmul(
                                        avp[:, col0:],
                                        v_sb[:, ktile, P * hh:P * (hh + 1)],
                                        et[:, col0:], start=(ktile == 0),
                                        stop=(ktile == nkt - 1))
                                rrep = stB.tile([P, CW], f32, tag="rrep")
                                nc.gpsimd.partition_broadcast(rrep[:], rec[:])
                                att = attp.tile([P, CW], CDT, tag="att")
                                nc.vector.tensor_mul(att[:], avp[:], rrep[:])
                                at_ts[(qc, hh)] = att

                        def wo_mm(c):
                          with nc.named_scope(f"wo_c{c}"):
                            for ot in range(DT):
                                pt = ps_wo.tile([P, CW], f32, tag="pwo")
                                for kt in range(NHC):
                                    nc.tensor.matmul(pt[:], wo_sb[ot][:, kt],
                                                     at_ts[(c, kt)][:],
                                                     start=(kt == 0),
                                                     stop=(kt == NHC - 1))
                                og = stB.tile([P, CW], CDT, tag="og")
                                nc.vector.tensor_copy(out=og[:], in_=pt[:])
                                nc.sync.dma_start(
                                    out=oT_cc[c][P * ot:P * (ot + 1), :],
                                    in_=og[:])
                            for kt in range(NHC):
                                del at_ts[(c, kt)]
                            nc.gpsimd.collective_compute(
                                "ReduceScatter", ADD, ins=[oT_cc[c][:]],
                                outs=[o_rs[c][:]], replica_groups=RG)

                        attn_chunk(0)
                        attn_chunk(1)
                        wo_mm(0)
                        attn_chunk(2)
                        wo_mm(1)
                        attn_chunk(3)
                        wo_mm(2)
                        h_block(0, hstB, xtpB, ps_hstB)
                        wo_mm(3)
                        h_block(1, hstB, xtpB, ps_hstB)

            # ====== stage D: FFN + chunked RS + residual ======
            with (
                tc.tile_pool(name="stDh", bufs=2) as stDh,
                tc.tile_pool(name="stDw", bufs=2) as stDw,
                tc.tile_pool(name="stDw2", bufs=3) as stDw2,
                tc.tile_pool(name="stDg", bufs=2) as stDg,
                tc.tile_pool(name="stDt", bufs=3) as stDt,
                tc.tile_pool(name="hstD", bufs=2) as hstD,
                tc.tile_pool(name="xtpD", bufs=4) as xtpD,
                tc.tile_pool(name="ps_f1", bufs=2, space="PSUM") as ps_f1,
                tc.tile_pool(name="ps_f3", bufs=2, space="PSUM") as ps_f3,
                tc.tile_pool(name="ps_w2", bufs=2, space="PSUM") as ps_w2,
                tc.tile_pool(name="ps_hstD", bufs=1, space="PSUM") as ps_hstD,
            ):
                def residual(c):
                    # RS(f_c) must be complete at this program position.
                    with nc.named_scope(f"res_c{c}"):
                        for i in range(4):
                            o_sb = stDt.tile([P, CW], CDT, tag="osb")
                            nc.gpsimd.dma_start(
                                out=o_sb[:],
                                in_=fo_rs[c][P * i:P * (i + 1), :])
                            out_sb = stDt.tile([P, CW], f32, tag="outsb")
                            nc.vector.tensor_add(out_sb[:], hT[i][:, ch(c)],
                                                 o_sb[:])
                            nc.sync.dma_start(
                                out=outT_s[P * i:P * (i + 1), ch(c)],
                                in_=out_sb[:])

                for c in range(NCH):
                    with nc.named_scope(f"ffn_c{c}"):
                        hn_sb = stDh.tile([P, DT, CW], CDT, tag="hn")
                        hv = hT_ag[c][:].rearrange(
                            "(r p k) s -> p r k s", p=P, k=NHC)
                        for q4 in range(4):
                            nc.gpsimd.dma_start(
                                out=hn_sb[:, 8 * q4:8 * (q4 + 1), :],
                                in_=hv[:, 2 * q4:2 * (q4 + 1), :, :])
                        # s2chain(c) right before the scale that consumes it
                        s2chain(c, hstD)
                        for kt in range(DT):
                            nc.vector.tensor_mul(hn_sb[:, kt, :],
                                                 hn_sb[:, kt, :],
                                                 s2rep[:, ch(c)])
                        g_sb = stDg.tile([P, FT, CW], CDT, tag="g")
                        for ft in range(FT):
                            w1t = stDw.tile([P, DT, P], CDT, tag="w1")
                            w3t = stDw.tile([P, DT, P], CDT, tag="w3")
                            nc.scalar.dma_start(out=w1t[:], in_=w_1[ft])
                            nc.sync.dma_start(out=w3t[:], in_=w_3[ft])
                            p1 = ps_f1.tile([P, CW], f32, tag="p1")
                            p3 = ps_f3.tile([P, CW], f32, tag="p3")
                            for kt in range(DT):
                                nc.tensor.matmul(p1[:], w1t[:, kt],
                                                 hn_sb[:, kt, :],
                                                 start=(kt == 0),
                                                 stop=(kt == DT - 1))
                            for kt in range(DT):
                                nc.tensor.matmul(p3[:], w3t[:, kt],
                                                 hn_sb[:, kt, :],
                                                 start=(kt == 0),
                                                 stop=(kt == DT - 1))
                            tsi = stDt.tile([P, CW], CDT, tag="tsi")
                            nc.scalar.activation(out=tsi[:], in_=p1[:],
                                                 func=SILU)
                            nc.vector.tensor_mul(g_sb[:, ft, :], tsi[:],
                                                 p3[:])
                        if c == 0:
                            h_block(2, hstD, xtpD, ps_hstD)
                            h_block(3, hstD, xtpD, ps_hstD)
                        else:
                            residual(c - 1)
                        for ot in range(32):
                            w2t = stDw2.tile([P, FT, P], CDT, tag="w2")
                            if ot % 2 == 0:
                                nc.scalar.dma_start(out=w2t[:], in_=w_2[ot])
                            else:
                                nc.sync.dma_start(out=w2t[:], in_=w_2[ot])
                            pt = ps_w2.tile([P, CW], f32, tag="pw2")
                            for ft in range(FT):
                                nc.tensor.matmul(pt[:], w2t[:, ft],
                                                 g_sb[:, ft, :],
                                                 start=(ft == 0),
                                                 stop=(ft == FT - 1))
                            og = stDt.tile([P, CW], CDT, tag="og")
                            if ot % 2 == 0:
                                nc.vector.tensor_copy(out=og[:], in_=pt[:])
                            else:
                                nc.scalar.copy(out=og[:], in_=pt[:])
                            nc.sync.dma_start(
                                out=foT_cc[c][P * ot:P * (ot + 1), :],
                                in_=og[:])
                        nc.gpsimd.collective_compute(
                            "ReduceScatter", ADD, ins=[foT_cc[c][:]],
                            outs=[fo_rs[c][:]], replica_groups=RG)
                        if c == NCH - 1:
                            residual(c)

    nc.compile()
    return nc


def _prep_inputs(x, freqs_cos, freqs_sin, mask, attn_norm_w, wq, wk, wv, wo,
                 ffn_norm_w, w1, w2, w3):
    """Host-side sharding + weight layout. Returns in_maps for 8 cores."""
    f32 = np.float32
    x2 = np.asarray(x, f32)[0]                     # [S, D]
    xT = np.ascontiguousarray(x2.T)                # [D, S]
    # SBUF-tile-ordered x: x_ch[c, p, kt, s] = xT[128*kt+p, 512*c+s]
    x_ch = np.ascontiguousarray(
        xT.astype(NP_CDT).reshape(DT, P, NCH, CW).transpose(2, 1, 0, 3))
    anw = np.asarray(attn_norm_w, f32)
    fnw = np.asarray(ffn_norm_w, f32)
    wq = np.asarray(wq, f32) * anw[None, :]
    wk = np.asarray(wk, f32) * anw[None, :]
    wv_e = np.asarray(wv, f32)
    wo = np.asarray(wo, f32)
    w1 = np.asarray(w1, f32) * fnw[None, :]
    w3 = np.asarray(w3, f32) * fnw[None, :]
    w2 = np.asarray(w2, f32)

    perm = np.concatenate([np.arange(0, HD, 2), np.arange(1, HD, 2)])

    cosT = np.ascontiguousarray(np.asarray(freqs_cos, f32).T)   # [64, S]
    sinT = np.ascontiguousarray(np.asarray(freqs_sin, f32).T)
    cos2 = np.concatenate([cosT, cosT], axis=0).astype(NP_CDT)  # [128, S]
    sinsg2 = np.concatenate([-sinT, sinT], axis=0).astype(NP_CDT)
    m = np.asarray(mask, f32)[0, 0]
    dmask = (np.ascontiguousarray(m[:P, :P].T) * f32(math.sqrt(HD))).astype(f32)

    def lhsT_tiles(wt, n_out_tiles, n_k_tiles):
        # wt: [K, Mout] -> [ot, p, kt, j] with [ot,p,kt,j] = wt[128*kt+p, 128*ot+j]
        a = wt.reshape(n_k_tiles, P, n_out_tiles, P)
        return np.ascontiguousarray(a.transpose(2, 1, 0, 3)).astype(NP_CDT)

    in_maps = []
    for r in range(CORES):
        ds = slice(DQ * r, DQ * (r + 1))
        wqT = wq[ds].T.copy()                      # [D, DQ]
        wkT = wk[ds].T.copy()
        for h in range(NHC):
            blk = slice(HD * h, HD * (h + 1))
            wqT[:, blk] = wqT[:, blk][:, perm]
            wkT[:, blk] = wkT[:, blk][:, perm]
        wqk = np.concatenate([lhsT_tiles(wqT, NHC, DT),
                              lhsT_tiles(wkT, NHC, DT)], axis=0)  # [8,P,DT,P]
        wvT = wv_e[ds].T.copy()                    # [D, DQ]
        w_v_l = np.ascontiguousarray(wvT.reshape(DT, P, DQ)).astype(NP_CDT)
        # wo ROW-sharded: contract own 512 attn dims, all 4096 out dims
        # w_o_l[ot, p, h, j] = wo[128*ot+j, 512*r + 128*h + p]
        a = np.ascontiguousarray(wo[:, ds].T)      # [512 d_own, 4096 o]
        w_o_l = np.ascontiguousarray(
            a.reshape(NHC, P, DT, P).transpose(2, 1, 0, 3)).astype(NP_CDT)
        fs = slice(FC * r, FC * (r + 1))
        w1s = np.zeros((FP, D), f32)
        w3s = np.zeros((FP, D), f32)
        w1s[:FC] = w1[fs]
        w3s[:FC] = w3[fs]
        w1_l = lhsT_tiles(np.ascontiguousarray(w1s.T), FT, DT)  # [FT, P, DT, P]
        w3_l = lhsT_tiles(np.ascontiguousarray(w3s.T), FT, DT)
        w2s = np.zeros((FP, D), f32)
        w2s[:FC] = w2[:, fs].T                     # [FP, D] (rows = f)
        w2_l = lhsT_tiles(w2s, 32, FT)             # [32, P, FT, P]

        in_maps.append({
            "xT_s": np.ascontiguousarray(xT[ds]),
            "x_ch": x_ch,
            "w_qk": wqk,
            "w_v": w_v_l,
            "w_o": w_o_l,
            "w_1": w1_l,
            "w_3": w3_l,
            "w_2": w2_l,
            "cos2": cos2,
            "sinsg2": sinsg2,
            "dmask": dmask,
        })
    return in_maps


def kernel(x, freqs_cos, freqs_sin, mask, attn_norm_w, wq, wk, wv, wo,
           ffn_norm_w, w1, w2, w3, _trace=False):
    global _COMPILED
    if _COMPILED is None:
        _COMPILED = _build()
    nc = _COMPILED
    in_maps = _prep_inputs(x, freqs_cos, freqs_sin, mask, attn_norm_w,
                           wq, wk, wv, wo, ffn_norm_w, w1, w2, w3)
    res = run_bass_kernel_spmd(nc, in_maps, list(range(CORES)), trace=_trace)
    kernel.last_result = res
    outT = np.concatenate([res.results[r]["outT_s"] for r in range(CORES)],
                          axis=0)                  # [D, S]
    return np.ascontiguousarray(outT.T)[None].astype(np.float32)


# revision 28
# speedup vs baseline: 1.0478x; 1.0478x over previous
"""Llama-style transformer block on 8 TRN2 NeuronCores.

v5: skew-tolerant scheduling.  The machine's FIFO queues (scalar/sync/
gpsimd engine queues, cc queue) must never sit waiting on a collective
whose peers are still computing -- everything that CONSUMES a collective
result is emitted at a program point whose wall-clock is safely after the
collective completes, even with ~30us cross-core skew.
  - Stage A: per-chunk x-stats partials (own 512 dims, 4 ones-matmuls)
    + tiny AllReduce; x quarters on gpsimd queue, wqk alternated over
    scalar/sync queues (per-queue DMA bandwidth is the stage A pacer).
  - Attention: scores/rowsum staggered one ktile apart; AV chain after the
    rowsum chain so recip/broadcast hide under it; og casts on DVE only.
  - wo ROW-sharded (no attnT AllGather); wo_mm(c) -> RS(o_c).
  - h_block(c) (RS readback + h + h^2 stats matmuls + AG(h) + AR(ssh))
    and s2chain(c) (AR readback -> sqrt -> recip -> broadcast) are
    scheduled 1-2 chunks behind the RS/AR they consume.
  - FFN: hn pre-scaled once (bf16), silu straight from PSUM,
    g = silu(p1)*p3; weight loads split across scalar+sync queues.
Program order: A0..A3 | B0 B1 wo0 B2 wo1 h0 B3 wo2 h1 wo3 h2 s2c0 |
ffn0[hn,scale,s2c1,ft-loop,h3,s2c2,w2,RS,res] ffn1[...,s2c3,...] ffn2 ffn3
"""

import math

import ml_dtypes
import numpy as np

import concourse.bass as bass
import concourse.mybir as mybir
import concourse.tile as tile
from concourse import bacc
from concourse.bass_utils import run_bass_kernel_spmd

S = 2048
D = 4096
HD = 128
NH = 32
F = 11008
CORES = 8
NHC = NH // CORES          # heads per core = 4
DQ = NHC * HD              # q/k/v dims per core = 512
FC = F // CORES            # ffn dims per core = 1376
FT = 11                    # padded f-tiles per core
FP = FT * 128
EPS = 1e-5
P = 128
NCH = 4                    # 512-token chunks
CW = S // NCH              # chunk width = 512
DT = D // P                # d tiles = 32
ST = S // P                # s tiles = 16

CDT = mybir.dt.bfloat16
NP_CDT = ml_dtypes.bfloat16

_COMPILED = None


def _build():
    nc = bacc.Bacc("TRN2", target_bir_lowering=False, debug=False,
                   num_devices=CORES)
    f32 = mybir.dt.float32

    # ---- kernel I/O ----
    xT_s = nc.declare_dram_parameter("xT_s", [DQ, S], f32, isOutput=False)
    x_ch = nc.declare_dram_parameter("x_ch", [NCH, P, DT, CW], CDT,
                                     isOutput=False)
    w_qk = nc.declare_dram_parameter("w_qk", [8, P, DT, P], CDT, isOutput=False)
    w_v = nc.declare_dram_parameter("w_v", [P, DT, DQ], CDT, isOutput=False)
    w_o = nc.declare_dram_parameter("w_o", [P, DT, NHC, P], CDT, isOutput=False)
    w_1 = nc.declare_dram_parameter("w_1", [FT, P, DT, P], CDT, isOutput=False)
    w_3 = nc.declare_dram_parameter("w_3", [FT, P, DT, P], CDT, isOutput=False)
    w_2 = nc.declare_dram_parameter("w_2", [32, P, FT, P], CDT, isOutput=False)
    cos2 = nc.declare_dram_parameter("cos2", [P, S], CDT, isOutput=False)
    sinsg2 = nc.declare_dram_parameter("sinsg2", [P, S], CDT, isOutput=False)
    dmask = nc.declare_dram_parameter("dmask", [P, P], f32, isOutput=False)
    outT_s = nc.declare_dram_parameter("outT_s", [DQ, S], f32, isOutput=True)

    # ---- internal DRAM ----
    s1row = nc.dram_tensor("s1row", [1, S], f32)
    ssx_p = [nc.dram_tensor(f"ssx_p{c}", [1, CW], f32) for c in range(NCH)]
    ssx_a = [nc.dram_tensor(f"ssx_a{c}", [1, CW], f32) for c in range(NCH)]
    ssh_p = [nc.dram_tensor(f"ssh_p{c}", [1, CW], f32) for c in range(NCH)]
    ssh_a = [nc.dram_tensor(f"ssh_a{c}", [1, CW], f32) for c in range(NCH)]
    oT_cc = [nc.dram_tensor(f"oT_cc{c}", [D, CW], CDT) for c in range(NCH)]
    o_rs = [nc.dram_tensor(f"o_rs{c}", [DQ, CW], CDT) for c in range(NCH)]
    h_cc = [nc.dram_tensor(f"h_cc{c}", [DQ, CW], CDT) for c in range(NCH)]
    hT_ag = [nc.dram_tensor(f"hT_ag{c}", [D, CW], CDT, addr_space="Shared")
             for c in range(NCH)]
    foT_cc = [nc.dram_tensor(f"foT_cc{c}", [D, CW], CDT) for c in range(NCH)]
    fo_rs = [nc.dram_tensor(f"fo_rs{c}", [DQ, CW], CDT) for c in range(NCH)]

    RG = [list(range(CORES))]
    ADD = mybir.AluOpType.add
    BYP = mybir.AluOpType.bypass
    EXP = mybir.ActivationFunctionType.Exp
    SQRT = mybir.ActivationFunctionType.Sqrt
    SILU = mybir.ActivationFunctionType.Silu
    ISQ = 1.0 / math.sqrt(HD)

    def ch(c):
        return slice(CW * c, CW * (c + 1))

    with tile.TileContext(nc) as tc:
        with (
            tc.tile_pool(name="persist", bufs=1) as persist,
        ):
            ones = persist.tile([P, 1], CDT)
            nc.vector.memset(ones[:], 1.0)
            eps_sb = persist.tile([P, 1], f32)
            nc.vector.memset(eps_sb[:], EPS)
            dmask_sb = persist.tile([P, P], f32)
            nc.gpsimd.dma_start(out=dmask_sb[:], in_=dmask[:])
            hT = [persist.tile([P, S], CDT, tag=f"hT{i}", name=f"hT{i}")
                  for i in range(4)]
            s2rep = persist.tile([P, S], CDT)
            s1tok = persist.tile([P, ST], f32)

            with tc.tile_pool(name="qkvsb", bufs=1) as qkvsb:
                qts = [qkvsb.tile([P, S], CDT, tag=f"qt{h}", name=f"qt{h}")
                       for h in range(NHC)]
                kts = [qkvsb.tile([P, S], CDT, tag=f"kt{h}", name=f"kt{h}")
                       for h in range(NHC)]
                v_sb = qkvsb.tile([P, ST, DQ], CDT)

                # ======== stage A: local stats + Q/K/V (+RoPE) ========
                with (
                    tc.tile_pool(name="tbl", bufs=1) as tbl,
                    tc.tile_pool(name="xst1", bufs=1) as xst1,
                    tc.tile_pool(name="xst", bufs=2) as xst,
                    tc.tile_pool(name="stAx", bufs=5) as stAx,
                    tc.tile_pool(name="stAw", bufs=2) as stAw,
                    tc.tile_pool(name="wvp", bufs=1) as wvp,
                    tc.tile_pool(name="rope", bufs=2) as rope,
                    tc.tile_pool(name="ps_qkv", bufs=3, space="PSUM") as ps_qkv,
                    tc.tile_pool(name="ps_v", bufs=1, space="PSUM") as ps_v,
                    tc.tile_pool(name="ps_xst", bufs=1, space="PSUM") as ps_xst,
                ):
                    # chunk-0 x quarters first so the first matmul is early
                    xq_c0 = [stAx.tile([P, 8, CW], CDT, tag="xq",
                                       name=f"xq{j}_0") for j in range(4)]
                    for j in range(4):
                        nc.gpsimd.dma_start(
                            out=xq_c0[j][:], in_=x_ch[0][:, 8 * j:8 * (j + 1), :])
                    cos_raw = tbl.tile([P, S], CDT, tag="cosr")
                    sin_raw = tbl.tile([P, S], CDT, tag="sinr")
                    nc.gpsimd.dma_start(out=cos_raw[:], in_=cos2[:])
                    nc.gpsimd.dma_start(out=sin_raw[:], in_=sinsg2[:])
                    wv_sb = wvp.tile([P, DT, DQ], CDT)

                    tables = {}

                    def stats_block(c):
                        # partial ssq over own 512 dims -> tiny AllReduce
                        xsl = xst1.tile([P, 4, CW], f32, tag="xsl",
                                        name=f"xsl{c}")
                        for i in range(4):
                            nc.gpsimd.dma_start(
                                out=xsl[:, i, :],
                                in_=xT_s[P * i:P * (i + 1), ch(c)])
                        sqx = xst1.tile([P, 4, CW], CDT, tag="sqx",
                                        name=f"sqx{c}")
                        nc.vector.tensor_mul(sqx[:], xsl[:], xsl[:])
                        pst = ps_xst.tile([1, CW], f32, tag="pst")
                        for i in range(4):
                            nc.tensor.matmul(pst[:], ones[:], sqx[:, i, :],
                                             start=(i == 0), stop=(i == 3))
                        row = xst.tile([1, CW], f32, tag="xrow",
                                       name=f"xrow{c}")
                        nc.scalar.copy(out=row[:], in_=pst[:])
                        nc.gpsimd.dma_start(out=ssx_p[c][:], in_=row[:])
                        nc.gpsimd.collective_compute(
                            "AllReduce", ADD, ins=[ssx_p[c][:]],
                            outs=[ssx_a[c][:]], replica_groups=RG)
                        rowa = xst.tile([1, CW], f32, tag="xrowa",
                                        name=f"xrowa{c}")
                        nc.gpsimd.dma_start(out=rowa[:], in_=ssx_a[c][:])
                        nc.scalar.activation(out=rowa[:], in_=rowa[:],
                                             func=SQRT, bias=eps_sb[0:1],
                                             scale=1.0 / D)
                        nc.vector.reciprocal(out=rowa[:], in_=rowa[:])
                        # token-major copy for the V scale
                        nc.gpsimd.dma_start(out=s1row[0:1, ch(c)], in_=rowa[:])
                        nc.gpsimd.dma_start(
                            out=s1tok[:, 4 * c:4 * c + 4],
                            in_=s1row[0:1, ch(c)].rearrange(
                                "o (j p) -> p (o j)", p=P))
                        s1b = xst.tile([1, CW], CDT, tag="s1b", name=f"s1b{c}")
                        nc.vector.tensor_copy(out=s1b[:], in_=rowa[:])
                        s1rep = xst.tile([P, CW], CDT, tag="s1rep",
                                         name=f"s1rep{c}")
                        nc.gpsimd.partition_broadcast(s1rep[:], s1b[:])
                        cs_t = rope.tile([P, CW], CDT, tag="cs", name=f"cs{c}")
                        sn_t = rope.tile([P, CW], CDT, tag="sn", name=f"sn{c}")
                        nc.vector.tensor_mul(cs_t[:], cos_raw[:, ch(c)],
                                             s1rep[:])
                        nc.vector.tensor_mul(sn_t[:], sin_raw[:, ch(c)],
                                             s1rep[:])
                        tables[c] = (cs_t, sn_t)

                    stats_block(0)
                    nc.gpsimd.dma_start(out=wv_sb[:], in_=w_v[:])

                    for c in range(NCH):
                      with nc.named_scope(f"qkv_c{c}"):
                        if c == 0:
                            xq = xq_c0
                        else:
                            xq = [stAx.tile([P, 8, CW], CDT, tag="xq",
                                            name=f"xq{j}_{c}")
                                  for j in range(4)]
                            for j in range(4):
                                nc.gpsimd.dma_start(
                                    out=xq[j][:],
                                    in_=x_ch[c][:, 8 * j:8 * (j + 1), :])

                        def xkt(kt):
                            return xq[kt // 8][:, kt % 8, :]

                        cs_t, sn_t = tables.pop(c)
                        if c + 1 < NCH:
                            stats_block(c + 1)  # pipelined under this chunk

                        # --- Q and K projections + RoPE ---
                        for ot in range(8):
                            wt = stAw.tile([P, DT, P], CDT, tag="wqk")
                            if ot % 2 == 0:
                                nc.scalar.dma_start(out=wt[:], in_=w_qk[ot])
                            else:
                                nc.sync.dma_start(out=wt[:], in_=w_qk[ot])
                            pt = ps_qkv.tile([P, CW], f32, tag="pqk")
                            for kt in range(DT):
                                nc.tensor.matmul(pt[:], wt[:, kt], xkt(kt),
                                                 start=(kt == 0),
                                                 stop=(kt == DT - 1))
                            swp = rope.tile([P, CW], f32, tag="swp")
                            nc.vector.tensor_copy(swp[0:64, :], pt[64:128, :])
                            nc.vector.tensor_copy(swp[64:128, :], pt[0:64, :])
                            t1 = rope.tile([P, CW], f32, tag="t1")
                            nc.vector.tensor_mul(t1[:], pt[:], cs_t[:])
                            nc.vector.tensor_mul(swp[:], swp[:], sn_t[:])
                            dst = qts[ot % 4] if ot < 4 else kts[ot % 4]
                            nc.vector.tensor_add(dst[:, ch(c)], t1[:], swp[:])

                        # --- V: 4 token-tiles of this chunk ---
                        pts = [ps_v.tile([P, DQ], f32, tag=f"pv{i}",
                                         name=f"pv{i}") for i in range(4)]
                        for kt in range(DT):
                            for i in range(4):
                                tok = slice(P * i, P * (i + 1))
                                nc.tensor.matmul(
                                    pts[i][:], xkt(kt)[:, tok],
                                    wv_sb[:, kt, :],
                                    start=(kt == 0), stop=(kt == DT - 1))
                        for i in range(4):
                            st = 4 * c + i
                            nc.vector.tensor_scalar_mul(
                                out=v_sb[:, st, :], in0=pts[i][:],
                                scalar1=s1tok[:, st:st + 1])

                def h_block(c, hst, xtp, ps_hst):
                    # RS(o_c) must be complete (with skew margin) at the
                    # wall-clock of this block's program position.
                    with nc.named_scope(f"h_c{c}"):
                        psh = ps_hst.tile([1, CW], f32, tag="psh")
                        for i in range(4):
                            osb = hst.tile([P, CW], CDT, tag="osb")
                            nc.gpsimd.dma_start(
                                out=osb[:], in_=o_rs[c][P * i:P * (i + 1), :])
                            xt = xtp.tile([P, CW], f32, tag="xt")
                            nc.sync.dma_start(
                                out=xt[:], in_=xT_s[P * i:P * (i + 1), ch(c)])
                            nc.vector.tensor_add(hT[i][:, ch(c)], xt[:],
                                                 osb[:])
                            nc.gpsimd.dma_start(
                                out=h_cc[c][:].rearrange(
                                    "(p k) s -> p k s", p=P)[:, i, :],
                                in_=hT[i][:, ch(c)])
                            sq = hst.tile([P, CW], CDT, tag="sq")
                            nc.vector.tensor_mul(sq[:], hT[i][:, ch(c)],
                                                 hT[i][:, ch(c)])
                            nc.tensor.matmul(psh[:], ones[:], sq[:],
                                             start=(i == 0), stop=(i == 3))
                        nc.gpsimd.collective_compute(
                            "AllGather", BYP, ins=[h_cc[c][:]],
                            outs=[hT_ag[c][:]], replica_groups=RG)
                        hrow = hst.tile([1, CW], f32, tag="hrow")
                        nc.scalar.copy(out=hrow[:], in_=psh[:])
                        nc.gpsimd.dma_start(out=ssh_p[c][:], in_=hrow[:])
                        nc.gpsimd.collective_compute(
                            "AllReduce", ADD, ins=[ssh_p[c][:]],
                            outs=[ssh_a[c][:]], replica_groups=RG)

                def s2chain(c, hst):
                    # AR(ssh_c) must be complete at this program position.
                    with nc.named_scope(f"s2_c{c}"):
                        rh = hst.tile([1, CW], f32, tag="s2r")
                        nc.gpsimd.dma_start(out=rh[:], in_=ssh_a[c][:])
                        nc.scalar.activation(out=rh[:], in_=rh[:], func=SQRT,
                                             bias=eps_sb[0:1], scale=1.0 / D)
                        nc.vector.reciprocal(out=rh[:], in_=rh[:])
                        s2b = hst.tile([1, CW], CDT, tag="s2b")
                        nc.vector.tensor_copy(out=s2b[:], in_=rh[:])
                        nc.gpsimd.partition_broadcast(s2rep[:, ch(c)], s2b[:])

                # ======== stage B: attention + row-sharded wo ========
                if True:
                    with (
                        tc.tile_pool(name="stB", bufs=4) as stB,
                        tc.tile_pool(name="exps", bufs=18) as exps,
                        tc.tile_pool(name="attp", bufs=8) as attp,
                        tc.tile_pool(name="woW", bufs=1) as woW,
                        tc.tile_pool(name="hstB", bufs=2) as hstB,
                        tc.tile_pool(name="xtpB", bufs=6) as xtpB,
                        tc.tile_pool(name="ps_sc", bufs=2,
                                     space="PSUM") as ps_sc,
                        tc.tile_pool(name="ps_av", bufs=2,
                                     space="PSUM") as ps_av,
                        tc.tile_pool(name="ps_sm", bufs=1,
                                     space="PSUM") as ps_sm,
                        tc.tile_pool(name="ps_wo", bufs=2,
                                     space="PSUM") as ps_wo,
                        tc.tile_pool(name="ps_hstB", bufs=1,
                                     space="PSUM") as ps_hstB,
                    ):
                        wo_sb = woW.tile([P, DT, NHC, P], CDT)
                        nc.sync.dma_start(out=wo_sb[:], in_=w_o[:])

                        at_ts = {}

                        def attn_chunk(qc):
                          with nc.named_scope(f"attn_c{qc}"):
                            nkt = 4 * qc + 4
                            for hh in range(NHC):
                                qt, kt_t = qts[hh], kts[hh]
                                smp = ps_sm.tile([1, CW], f32, tag="smp")
                                ets = []
                                for ktile in range(nkt):
                                    diag = ktile >= 4 * qc
                                    col0 = P * (ktile - 4 * qc) if diag else 0
                                    scp = ps_sc.tile([P, CW], f32, tag="scp")
                                    nc.tensor.matmul(
                                        scp[:, col0:],
                                        kt_t[:, P * ktile:P * (ktile + 1)],
                                        qt[:, CW * qc + col0:CW * (qc + 1)],
                                        start=True, stop=True)
                                    if diag:
                                        nc.vector.tensor_add(
                                            scp[:, col0:col0 + P],
                                            scp[:, col0:col0 + P],
                                            dmask_sb[:])
                                    et = exps.tile([P, CW], CDT, tag="et")
                                    nc.scalar.activation(out=et[:, col0:],
                                                         in_=scp[:, col0:],
                                                         func=EXP, scale=ISQ)
                                    ets.append((et, col0))
                                    if ktile > 0:
                                        pe, pc0 = ets[ktile - 1]
                                        nc.tensor.matmul(
                                            smp[:, pc0:], ones[:],
                                            pe[:, pc0:],
                                            start=(ktile == 1), stop=False)
                                pe, pc0 = ets[nkt - 1]
                                nc.tensor.matmul(smp[:, pc0:], ones[:],
                                                 pe[:, pc0:],
                                                 start=(nkt == 1), stop=True)
                                rec = stB.tile([1, CW], f32, tag="rec")
                                nc.vector.reciprocal(out=rec[:], in_=smp[:])
                                avp = ps_av.tile([P, CW], f32, tag="avp")
                                for ktile in range(nkt):
                                    et, col0 = ets[ktile]
                                    nc.tensor.matmul(
                                        avp[:, col0:],
                                        v_sb[:, ktile, P * hh:P * (hh + 1)],
                                        et[:, col0:], start=(ktile == 0),
                                        stop=(ktile == nkt - 1))
                                rrep = stB.tile([P, CW], f32, tag="rrep")
                                nc.gpsimd.partition_broadcast(rrep[:], rec[:])
                                att = attp.tile([P, CW], CDT, tag="att")
                                nc.vector.tensor_mul(att[:], avp[:], rrep[:])
                                at_ts[(qc, hh)] = att

                        def wo_mm(c):
                          with nc.named_scope(f"wo_c{c}"):
                            for ot in range(DT):
                                pt = ps_wo.tile([P, CW], f32, tag="pwo")
                                for kt in range(NHC):
                                    nc.tensor.matmul(pt[:],
                                                     wo_sb[:, ot, kt, :],
                                                     at_ts[(c, kt)][:],
                                                     start=(kt == 0),
                                                     stop=(kt == NHC - 1))
                                og = stB.tile([P, CW], CDT, tag="og")
                                nc.vector.tensor_copy(out=og[:], in_=pt[:])
                                nc.sync.dma_start(
                                    out=oT_cc[c][P * ot:P * (ot + 1), :],
                                    in_=og[:])
                            for kt in range(NHC):
                                del at_ts[(c, kt)]
                            nc.gpsimd.collective_compute(
                                "ReduceScatter", ADD, ins=[oT_cc[c][:]],
                                outs=[o_rs[c][:]], replica_groups=RG)

                        attn_chunk(0)
                        attn_chunk(1)
                        wo_mm(0)
                        attn_chunk(2)
                        wo_mm(1)
                        attn_chunk(3)
                        wo_mm(2)
                        h_block(0, hstB, xtpB, ps_hstB)
                        wo_mm(3)
                        h_block(1, hstB, xtpB, ps_hstB)

            # ====== stage D: FFN + chunked RS + residual ======
            with (
                tc.tile_pool(name="stDh", bufs=2) as stDh,
                tc.tile_pool(name="stDw", bufs=2) as stDw,
                tc.tile_pool(name="stDw2", bufs=3) as stDw2,
                tc.tile_pool(name="stDg", bufs=2) as stDg,
                tc.tile_pool(name="stDt", bufs=3) as stDt,
                tc.tile_pool(name="hstD", bufs=2) as hstD,
                tc.tile_pool(name="xtpD", bufs=4) as xtpD,
                tc.tile_pool(name="ps_f1", bufs=2, space="PSUM") as ps_f1,
                tc.tile_pool(name="ps_f3", bufs=2, space="PSUM") as ps_f3,
                tc.tile_pool(name="ps_w2", bufs=2, space="PSUM") as ps_w2,
                tc.tile_pool(name="ps_hstD", bufs=1, space="PSUM") as ps_hstD,
            ):
                def residual(c):
                    # RS(f_c) must be complete at this program position.
                    with nc.named_scope(f"res_c{c}"):
                        for i in range(4):
                            o_sb = stDt.tile([P, CW], CDT, tag="osb")
                            nc.gpsimd.dma_start(
                                out=o_sb[:],
                                in_=fo_rs[c][P * i:P * (i + 1), :])
                            out_sb = stDt.tile([P, CW], f32, tag="outsb")
                            nc.vector.tensor_add(out_sb[:], hT[i][:, ch(c)],
                                                 o_sb[:])
                            nc.sync.dma_start(
                                out=outT_s[P * i:P * (i + 1), ch(c)],
                                in_=out_sb[:])

                for c in range(NCH):
                    with nc.named_scope(f"ffn_c{c}"):
                        hn_sb = stDh.tile([P, DT, CW], CDT, tag="hn")
                        # (k s) merged so each (p, r) is a 4KB contiguous run
                        hv = hT_ag[c][:].rearrange(
                            "(r p k) s -> p r (k s)", p=P, k=NHC)
                        for q4 in range(4):
                            nc.gpsimd.dma_start(
                                out=hn_sb[:, 8 * q4:8 * (q4 + 1), :].rearrange(
                                    "p k s -> p (k s)"),
                                in_=hv[:, 2 * q4:2 * (q4 + 1), :])
                        # s2chain(c) right before the scale that consumes it
                        s2chain(c, hstD)
                        for kt in range(DT):
                            nc.vector.tensor_mul(hn_sb[:, kt, :],
                                                 hn_sb[:, kt, :],
                                                 s2rep[:, ch(c)])
                        g_sb = stDg.tile([P, FT, CW], CDT, tag="g")
                        for ft in range(FT):
                            w1t = stDw.tile([P, DT, P], CDT, tag="w1")
                            w3t = stDw.tile([P, DT, P], CDT, tag="w3")
                            nc.scalar.dma_start(out=w1t[:], in_=w_1[ft])
                            nc.sync.dma_start(out=w3t[:], in_=w_3[ft])
                            p1 = ps_f1.tile([P, CW], f32, tag="p1")
                            p3 = ps_f3.tile([P, CW], f32, tag="p3")
                            for kt in range(DT):
                                nc.tensor.matmul(p1[:], w1t[:, kt],
                                                 hn_sb[:, kt, :],
                                                 start=(kt == 0),
                                                 stop=(kt == DT - 1))
                            for kt in range(DT):
                                nc.tensor.matmul(p3[:], w3t[:, kt],
                                                 hn_sb[:, kt, :],
                                                 start=(kt == 0),
                                                 stop=(kt == DT - 1))
                            tsi = stDt.tile([P, CW], CDT, tag="tsi")
                            nc.scalar.activation(out=tsi[:], in_=p1[:],
                                                 func=SILU)
                            nc.vector.tensor_mul(g_sb[:, ft, :], tsi[:],
                                                 p3[:])
                        if c == 0:
                            h_block(2, hstD, xtpD, ps_hstD)
                            h_block(3, hstD, xtpD, ps_hstD)
                        else:
                            residual(c - 1)
                        for ot in range(32):
                            w2t = stDw2.tile([P, FT, P], CDT, tag="w2")
                            if ot % 2 == 0:
                                nc.scalar.dma_start(out=w2t[:], in_=w_2[ot])
                            else:
                                nc.sync.dma_start(out=w2t[:], in_=w_2[ot])
                            pt = ps_w2.tile([P, CW], f32, tag="pw2")
                            for ft in range(FT):
                                nc.tensor.matmul(pt[:], w2t[:, ft],
                                                 g_sb[:, ft, :],
                                                 start=(ft == 0),
                                                 stop=(ft == FT - 1))
                            og = stDt.tile([P, CW], CDT, tag="og")
                            if ot % 2 == 0:
                                nc.vector.tensor_copy(out=og[:], in_=pt[:])
                            else:
                                nc.scalar.copy(out=og[:], in_=pt[:])
                            nc.sync.dma_start(
                                out=foT_cc[c][P * ot:P * (ot + 1), :],
                                in_=og[:])
                        nc.gpsimd.collective_compute(
                            "ReduceScatter", ADD, ins=[foT_cc[c][:]],
                            outs=[fo_rs[c][:]], replica_groups=RG)
                        if c == NCH - 1:
                            residual(c)

    nc.compile()
    return nc


def _prep_inputs(x, freqs_cos, freqs_sin, mask, attn_norm_w, wq, wk, wv, wo,
                 ffn_norm_w, w1, w2, w3):
    """Host-side sharding + weight layout. Returns in_maps for 8 cores."""
    f32 = np.float32
    x2 = np.asarray(x, f32)[0]                     # [S, D]
    xT = np.ascontiguousarray(x2.T)                # [D, S]
    # SBUF-tile-ordered x: x_ch[c, p, kt, s] = xT[128*kt+p, 512*c+s]
    x_ch = np.ascontiguousarray(
        xT.astype(NP_CDT).reshape(DT, P, NCH, CW).transpose(2, 1, 0, 3))
    anw = np.asarray(attn_norm_w, f32)
    fnw = np.asarray(ffn_norm_w, f32)
    wq = np.asarray(wq, f32) * anw[None, :]
    wk = np.asarray(wk, f32) * anw[None, :]
    wv_e = np.asarray(wv, f32)
    wo = np.asarray(wo, f32)
    w1 = np.asarray(w1, f32) * fnw[None, :]
    w3 = np.asarray(w3, f32) * fnw[None, :]
    w2 = np.asarray(w2, f32)

    perm = np.concatenate([np.arange(0, HD, 2), np.arange(1, HD, 2)])

    cosT = np.ascontiguousarray(np.asarray(freqs_cos, f32).T)   # [64, S]
    sinT = np.ascontiguousarray(np.asarray(freqs_sin, f32).T)
    cos2 = np.concatenate([cosT, cosT], axis=0).astype(NP_CDT)  # [128, S]
    sinsg2 = np.concatenate([-sinT, sinT], axis=0).astype(NP_CDT)
    m = np.asarray(mask, f32)[0, 0]
    dmask = (np.ascontiguousarray(m[:P, :P].T) * f32(math.sqrt(HD))).astype(f32)

    def lhsT_tiles(wt, n_out_tiles, n_k_tiles):
        # wt: [K, Mout] -> [ot, p, kt, j] with [ot,p,kt,j] = wt[128*kt+p, 128*ot+j]
        a = wt.reshape(n_k_tiles, P, n_out_tiles, P)
        return np.ascontiguousarray(a.transpose(2, 1, 0, 3)).astype(NP_CDT)

    in_maps = []
    for r in range(CORES):
        ds = slice(DQ * r, DQ * (r + 1))
        wqT = wq[ds].T.copy()                      # [D, DQ]
        wkT = wk[ds].T.copy()
        for h in range(NHC):
            blk = slice(HD * h, HD * (h + 1))
            wqT[:, blk] = wqT[:, blk][:, perm]
            wkT[:, blk] = wkT[:, blk][:, perm]
        wqk = np.concatenate([lhsT_tiles(wqT, NHC, DT),
                              lhsT_tiles(wkT, NHC, DT)], axis=0)  # [8,P,DT,P]
        wvT = wv_e[ds].T.copy()                    # [D, DQ]
        # [P, DT, DQ]: 32KB contiguous per partition -> one efficient DMA
        w_v_l = np.ascontiguousarray(
            wvT.reshape(DT, P, DQ).transpose(1, 0, 2)).astype(NP_CDT)
        # wo ROW-sharded: contract own 512 attn dims, all 4096 out dims
        # w_o_l[p, ot, h, j] = wo[128*ot+j, 512*r + 128*h + p]
        a = np.ascontiguousarray(wo[:, ds].T)      # [512 d_own, 4096 o]
        w_o_l = np.ascontiguousarray(
            a.reshape(NHC, P, DT, P).transpose(1, 2, 0, 3)).astype(NP_CDT)
        fs = slice(FC * r, FC * (r + 1))
        w1s = np.zeros((FP, D), f32)
        w3s = np.zeros((FP, D), f32)
        w1s[:FC] = w1[fs]
        w3s[:FC] = w3[fs]
        w1_l = lhsT_tiles(np.ascontiguousarray(w1s.T), FT, DT)  # [FT, P, DT, P]
        w3_l = lhsT_tiles(np.ascontiguousarray(w3s.T), FT, DT)
        w2s = np.zeros((FP, D), f32)
        w2s[:FC] = w2[:, fs].T                     # [FP, D] (rows = f)
        w2_l = lhsT_tiles(w2s, 32, FT)             # [32, P, FT, P]

        in_maps.append({
            "xT_s": np.ascontiguousarray(xT[ds]),
            "x_ch": x_ch,
            "w_qk": wqk,
            "w_v": w_v_l,
            "w_o": w_o_l,
            "w_1": w1_l,
            "w_3": w3_l,
            "w_2": w2_l,
            "cos2": cos2,
            "sinsg2": sinsg2,
            "dmask": dmask,
        })
    return in_maps


def kernel(x, freqs_cos, freqs_sin, mask, attn_norm_w, wq, wk, wv, wo,
           ffn_norm_w, w1, w2, w3, _trace=False):
    global _COMPILED
    if _COMPILED is None:
        _COMPILED = _build()
    nc = _COMPILED
    in_maps = _prep_inputs(x, freqs_cos, freqs_sin, mask, attn_norm_w,
                           wq, wk, wv, wo, ffn_norm_w, w1, w2, w3)
    res = run_bass_kernel_spmd(nc, in_maps, list(range(CORES)), trace=_trace)
    kernel.last_result = res
    outT = np.concatenate([res.results[r]["outT_s"] for r in range(CORES)],
                          axis=0)                  # [D, S]
    return np.ascontiguousarray(outT.T)[None].astype(np.float32)
